# revision 9
# baseline (speedup 1.0000x reference)
"""Trainium2 Bass kernel for nn_AME2Encoder (dense_mlp, 8-core data parallel).

Strategy:
  - Pure data parallel: B=2048 sharded 256/core; each core processes its
    samples as 128 "pairs" (2 samples packed on the 128 SBUF partitions).
  - Feature-major bf16 activations ([feat, token] tiles, token tile = 504
    = one sample's full 14x36 grid). All matmuls keep this layout so no
    transposes are needed anywhere.
  - 64-feature layers are packed 2-samples-per-matmul with block-diagonal
    weights (M=128), or with PE tile_position quadrant packing.
  - conv1 (3x3) is a single K=54 matmul per pair over a host-prepared
    im2col layout (input layout prep; all FLOPs on device).
  - ELU: exact decomposition elu(x) = min(exp(x),1)-1+relu(x) on ACT+GPSIMD
    for conv1/conv2; single-pass fused custom DVE op (quadratic minimax fit
    of expm1 on the small negative range of the pre-activations) for
    fuse/g1/q.
  - Single-query MHA: scores for both samples of a pair via one PE wave of
    two quadrant-packed matmuls with per-sample stationary 0.5*Q masks;
    softmax exp on ACT with free accumulation; V-weighting on GPSIMD.
"""

import dataclasses
import os
from contextlib import ExitStack

import numpy as np
import ml_dtypes

import concourse.bass as bass
import concourse.mybir as mybir
import concourse.tile as tile
from concourse.bass_utils import run_bass_kernel_spmd
from concourse.vector_clock import ScopedClock


# --- workaround: this walrus rejects the tail Drain carrying >1 sem waits ---
def _patched_dab(self, tick_clock, wait_clock):
    nc = self.nc
    probe = nc.sync.drain()
    wait_clock.add_sem_waits(probe.ins, ScopedClock({None: tick_clock.global_clock}))
    si = probe.ins.sync_info
    waits = list(si.on_wait) if si is not None else []
    if si is not None and len(waits) > 1:
        si.on_wait = waits[:1]
        for w in waits[1:]:
            n2 = nc.sync.drain()
            n2.ins.sync_info = mybir.SyncInfo(on_wait=[w], on_update=[])
    nc.all_engine_barrier()
    assert self.sems is not None
    popped = nc._tile_sem_poison_stack.pop()
    assert popped is self._sem_poison
    nc.clear_and_free_semaphores(list(self.sems.allocated().values()))
    nc.all_engine_barrier()


tile.TileContext._drain_and_barrier = _patched_dab


def _split_multiwait(nc, max_waits=1):
    """This walrus build cannot encode >1 sem-wait on one instruction for some
    structs; hoist excess waits onto EventSemaphore carriers inserted before."""
    ctr = [0]
    for fn in nc.m.functions:
        for blk in fn.blocks:
            insts = list(blk.instructions)
            new = []
            changed = False
            for inst in insts:
                si = inst.sync_info
                waits = list(si.on_wait) if si is not None and si.on_wait else []
                if len(waits) > max_waits:
                    changed = True
                    for w in waits[max_waits:]:
                        ctr[0] += 1
                        new.append(mybir.InstEventSemaphore(
                            name=f"zz_mw_{ctr[0]}", engine=inst.engine,
                            ins=[], outs=[],
                            sync_info=mybir.SyncInfo(on_wait=[w], on_update=[]),
                        ))
                    inst.sync_info = mybir.SyncInfo(
                        on_wait=waits[:max_waits],
                        on_update=list(si.on_update) if si.on_update else [],
                    )
                new.append(inst)
            if changed:
                blk.instructions = new

# ----- problem constants (hardcoded per spec) -----
B, C_IN, H, W = 2048, 3, 14, 36
D_LOCAL, D_POS, D_GLOBAL, D_PROP, NH = 64, 64, 128, 128, 16
HD = D_LOCAL // NH
N_CORES = 8
B_LOC = B // N_CORES      # 256
NPAIR = B_LOC // 2        # 128
L = H * W                 # 504

BF = mybir.dt.bfloat16
F32 = mybir.dt.float32
bf16 = ml_dtypes.bfloat16
AX = mybir.AluOpType
AF = mybir.ActivationFunctionType

# ELU path config: which sites use the fused custom DVE op (others use the
# exact ACT exp/relu + GPSIMD tail).
ELU_CUSTOM = {"conv1": False, "conv2": False, "fuse": False, "g1": False, "q": False}
# quadratic expm1 fit range per custom site (pre-act min measured ~-1.2 worst)
ELU_RANGE = {"fuse": 0.5, "g1": 0.35}

# ----------------------------------------------------------------------------
# Custom fused ELU DVE op: out = relu(t) + m*(C1 + m*C2), t = Src0 + C0,
# m = min(t, 0).  C0 = per-partition bias AP, C1/C2 = quadratic fit of expm1.
# ----------------------------------------------------------------------------
_ELU_OP = None


def _register_elu_op():
    global _ELU_OP
    if _ELU_OP is not None:
        return _ELU_OP
    from concourse.dve_spec import (
        Spec, Src0, C0, C1, C2, Zero, relu, minn, lower, _has_src1,
    )
    from concourse.dve_uop import DveOpSpec
    from concourse import dve_ops
    from concourse.dve_ops import DveOp, OPS, _SUB_OPCODE_FOR_NAME, _CUSTOM_DVE_ROW_BASE

    name = "ELU_Q_ANT"
    if name in _SUB_OPCODE_FOR_NAME:
        _ELU_OP = next(o for o in OPS if o.name == name)
        return _ELU_OP

    t = Src0 + C0
    m = minn(t, Zero)
    body = relu(t) + m * (C1 + m * C2)

    def ref(in0, in1, s0, s1, imm2):
        tt = in0 + s0
        mm = np.minimum(tt, 0.0)
        return (np.maximum(tt, 0.0) + mm * (s1 + mm * imm2)).astype(np.float32)

    spec = Spec(body=body, reference=ref)
    op0 = DveOp(name, spec, subdim=False, uops_sha={})
    OPS.append(op0)
    _SUB_OPCODE_FOR_NAME[name] = _CUSTOM_DVE_ROW_BASE + len(OPS) - 1
    shas = {}
    for ver in ("v3", "v4"):
        u = lower(spec, ver=ver)
        shas[ver] = DveOpSpec(
            name=name, opcode=_SUB_OPCODE_FOR_NAME[name], uops=u,
            rd1_en=_has_src1(spec),
        ).sha(ver)
    op = dataclasses.replace(op0, uops_sha=shas)
    OPS[-1] = op
    _ELU_OP = op
    return op


def _fit_expm1_quad(R):
    """minimax-ish quadratic fit of expm1 on [-R, 0]: c1*m + c2*m^2."""
    m = np.linspace(-R, 0, 4001)
    y = np.expm1(m)
    A = np.stack([m, m * m], axis=1)
    w = np.ones_like(m)
    c = None
    for _ in range(60):
        c, *_ = np.linalg.lstsq(A * w[:, None], y * w, rcond=None)
        e = np.abs(A @ c - y)
        w = 0.9 * w + 0.1 * (1 + 10 * e / (e.max() + 1e-12))
    return float(c[0]), float(c[1])


_ELU_C = {site: _fit_expm1_quad(R) for site, R in ELU_RANGE.items()}


def _np_elu(x):
    return np.where(x > 0, x, np.expm1(np.minimum(x, 0.0)))


# ----------------------------------------------------------------------------
# Host-side constant packing (weight folding / layout prep)
# ----------------------------------------------------------------------------

def _block_diag2(w):
    """[k, m] -> [2k, 2m] block diagonal duplication."""
    k, m = w.shape
    out = np.zeros((2 * k, 2 * m), np.float32)
    out[:k, :m] = w
    out[k:, m:] = w
    return out


def _dup_col(b):
    return np.concatenate([b, b]).astype(np.float32)[:, None]


def host_prep_shared(inp):
    """Build the weight-derived dram parameters (shared across cores)."""
    c = {}
    # conv1 im2col weights: k = (3*dy+dx)*3 + cin
    w1p = inp["conv1_w"].transpose(2, 3, 1, 0).reshape(27, 64)
    c["w1bd"] = _block_diag2(w1p).astype(bf16)            # [54,128]
    c["b1d"] = _dup_col(inp["conv1_b"])                   # [128,1] f32
    w2 = inp["conv2_w"][:, :, 0, 0].T                     # [in,out]
    c["w2bd"] = _block_diag2(w2).astype(bf16)             # [128,128]
    c["b2d"] = _dup_col(inp["conv2_b"])
    fl = inp["fuse_w"][:D_LOCAL]                          # [64,64]
    fp = inp["fuse_w"][D_LOCAL:]                          # [64,64]
    c["wflbd"] = _block_diag2(fl).astype(bf16)            # [128,128]
    c["wfp2"] = np.concatenate([fp, fp], axis=1).astype(bf16)  # [64,128]
    c["bfd"] = _dup_col(inp["fuse_b"])
    # positional MLP constant (input-independent): [64, 504] bf16
    ys = np.linspace(-1.0, 1.0, H, dtype=np.float32)
    xs = np.linspace(-1.0, 1.0, W, dtype=np.float32)
    gy, gx = np.meshgrid(ys, xs, indexing="ij")
    coords = np.stack([gx, gy], axis=-1).reshape(L, 2)
    pe = _np_elu(coords @ inp["pe_w1"] + inp["pe_b1"]) @ inp["pe_w2"] + inp["pe_b2"]
    c["pec"] = np.ascontiguousarray(pe.T).astype(bf16)    # [64,504]
    c["g1w2"] = np.vstack([inp["g_w1"], inp["g_w1"]]).astype(bf16)  # [128,128]
    c["bg1d"] = inp["g_b1"].astype(np.float32)[:, None]   # [128,1]
    c["g2w"] = inp["g_w2"].astype(bf16)                   # [128,128]
    c["bg2"] = inp["g_b2"].astype(np.float32)[:, None]    # [128,1]
    c["wv2"] = np.vstack([inp["wv"], inp["wv"]]).astype(bf16)   # [128,64]
    c["wk2"] = np.vstack([inp["wk"], inp["wk"]]).astype(bf16)
    c["bvd"] = _dup_col(inp["bv"])
    c["bkd"] = _dup_col(inp["bk"])
    sm = np.zeros((64, 64), np.float32)
    for k in range(64):
        sm[k, (k // HD) * HD:(k // HD + 1) * HD] = 1.0 / np.sqrt(HD)
    c["csm2"] = np.vstack([sm, sm]).astype(bf16)          # [128,64] mask*0.5
    c["qpwg"] = inp["qp_w"][:D_GLOBAL].astype(bf16)       # [128,64]
    c["qpwp"] = inp["qp_w"][D_GLOBAL:].astype(bf16)       # [128,64]
    c["qpb"] = inp["qp_b"].astype(np.float32)[:, None]    # [64,1]
    c["wq"] = inp["wq"].astype(bf16)
    c["bq2"] = _dup_col(inp["bq"])                        # [128,1]
    c["wobd"] = _block_diag2(inp["wo"]).astype(bf16)      # [128,128]
    c["bod"] = _dup_col(inp["bo"])
    return c


def host_prep_percore(inp):
    """Per-core data params: im2col input layout + transposed prop_emb."""
    mf = inp["map_feat"].astype(np.float32)
    mp = np.zeros((B, 3, H + 2, W + 2), np.float32)
    mp[:, :, 1:H + 1, 1:W + 1] = mf
    from numpy.lib.stride_tricks import sliding_window_view
    sw = sliding_window_view(mp, (3, 3), axis=(2, 3))     # [B,3,14,36,3,3]
    ic = sw.transpose(0, 4, 5, 1, 2, 3).reshape(B, 27, L)  # k=(3dy+dx)*3+c
    ic = np.ascontiguousarray(ic).reshape(B // 2, 54, L).astype(bf16)
    prop = inp["prop_emb"].astype(np.float32)
    cores = []
    for ci in range(N_CORES):
        sl = slice(ci * B_LOC, (ci + 1) * B_LOC)
        cores.append({
            "ic": np.ascontiguousarray(ic[ci * NPAIR:(ci + 1) * NPAIR]),
            "propT": np.ascontiguousarray(prop[sl].T).astype(bf16),  # [128,256]
        })
    return cores


# ----------------------------------------------------------------------------
# Bass graph
# ----------------------------------------------------------------------------

def build_nc(shared):
    elu_op = _register_elu_op()
    nc = bass.Bass()

    # dram parameters
    P = {}
    P["ic"] = nc.declare_dram_parameter("ic", [NPAIR, 54, L], BF, isOutput=False)
    P["propT"] = nc.declare_dram_parameter("propT", [D_PROP, B_LOC], BF, isOutput=False)
    for name, arr in shared.items():
        dt = BF if arr.dtype == bf16 else F32
        P[name] = nc.declare_dram_parameter(name, list(arr.shape), dt, isOutput=False)
    out_h = nc.declare_dram_parameter("out", [B_LOC, D_LOCAL + D_GLOBAL], F32, isOutput=True)

    def dram_ap(h, offset, dims):
        base = h[:]
        return bass.AP(tensor=base.tensor, offset=offset, ap=[list(d) for d in dims])

    with tile.TileContext(nc) as tc, ExitStack() as ctx:
        singles = ctx.enter_context(tc.tile_pool(name="singles", bufs=1))
        pA_in = ctx.enter_context(tc.tile_pool(name="pA_in", bufs=3))
        pA_sb = ctx.enter_context(tc.tile_pool(name="pA_sb", bufs=2))
        pC_sb = ctx.enter_context(tc.tile_pool(name="pC_sb", bufs=2))
        psA = ctx.enter_context(tc.tile_pool(name="psA", bufs=3, space="PSUM"))
        psG = ctx.enter_context(tc.tile_pool(name="psG", bufs=2, space="PSUM"))
        psC = ctx.enter_context(tc.tile_pool(name="psC", bufs=3, space="PSUM"))

        # ---- load constants ----
        cs = {}
        for name, arr in shared.items():
            dt = BF if arr.dtype == bf16 else F32
            t = singles.tile(list(arr.shape), dt, tag=f"c_{name}", name=f"c_{name}")
            nc.sync.dma_start(out=t[:], in_=P[name][:])
            cs[name] = t
        cprop = singles.tile([D_PROP, B_LOC], BF, tag="c_prop")
        nc.sync.dma_start(out=cprop[:], in_=P["propT"][:])

        # persistent state
        gf_all = singles.tile([D_GLOBAL, B_LOC], F32, tag="gf_all")
        gf_bf = singles.tile([D_GLOBAL, B_LOC], BF, tag="gf_bf")
        ctx_all = singles.tile([128, NPAIR], BF, tag="ctx_all")
        Q_sb = singles.tile([128, B_LOC], F32, tag="Q_sb")
        pw_tiles = [singles.tile([128, L], BF, tag=f"pw{j}", name=f"pw{j}")
                    for j in range(NPAIR)]

        def elu_custom(dst_ap, src_ap, bias_tile, site):
            c1, c2 = _ELU_C[site]
            nc.vector._custom_dve(
                elu_op, out=dst_ap, in0=src_ap, s0=bias_tile[:], s1=c1, imm2=c2)

        def elu_exact(pool, dst_ap, src_ap, bias_tile, nparts):
            # exact: elu(x+b) = min(exp(x+b),1) - 1 + relu(x+b)
            e = pool.tile([nparts, src_ap.shape[-1]], BF, tag="elu_e", name="elu_e")
            r = pool.tile([nparts, src_ap.shape[-1]], BF, tag="elu_r", name="elu_r")
            f = pool.tile([nparts, src_ap.shape[-1]], BF, tag="elu_f", name="elu_f")
            nc.scalar.activation(e[:], src_ap, AF.Exp, bias=bias_tile[:], scale=1.0)
            nc.scalar.activation(r[:], src_ap, AF.Relu, bias=bias_tile[:], scale=1.0)
            nc.gpsimd.tensor_scalar(f[:], e[:], 1.0, -1.0, op0=AX.min, op1=AX.add)
            nc.gpsimd.tensor_tensor(dst_ap, f[:], r[:], op=AX.add)

        # ================= PHASE A: conv/fuse/global per pair =================
        for j in range(NPAIR):
            ict = pA_in.tile([54, L], BF, tag="ict")
            nc.sync.dma_start(out=ict[:], in_=P["ic"][j])

            c1p = psA.tile([128, L], F32, tag="pa")
            nc.tensor.matmul(c1p[:], cs["w1bd"][:], ict[:], start=True, stop=True)
            a1 = pA_sb.tile([128, L], BF, tag="a1")
            if ELU_CUSTOM["conv1"]:
                elu_custom(a1[:], c1p[:], cs["b1d"], "conv1")
            else:
                elu_exact(pA_sb, a1[:], c1p[:], cs["b1d"], 128)

            c2p = psA.tile([128, L], F32, tag="pa")
            nc.tensor.matmul(c2p[:], cs["w2bd"][:], a1[:], start=True, stop=True)
            a2 = pA_sb.tile([128, L], BF, tag="a2")
            if ELU_CUSTOM["conv2"]:
                elu_custom(a2[:], c2p[:], cs["b2d"], "conv2")
            else:
                elu_exact(pA_sb, a2[:], c2p[:], cs["b2d"], 128)

            fp_ = psA.tile([128, L], F32, tag="pa")
            nc.tensor.matmul(fp_[:], cs["wflbd"][:], a2[:], start=True, stop=False)
            nc.tensor.matmul(fp_[:], cs["wfp2"][:], cs["pec"][:], start=False, stop=True)
            pwj = pw_tiles[j]
            if ELU_CUSTOM["fuse"]:
                elu_custom(pwj[:], fp_[:], cs["bfd"], "fuse")
            else:
                elu_exact(pA_sb, pwj[:], fp_[:], cs["bfd"], 128)

            for s in (0, 1):
                g1p = psG.tile([128, L], F32, tag="pg")
                nc.tensor.matmul(g1p[:], cs["g1w2"][64 * s:64 * (s + 1), :],
                                 pwj[64 * s:64 * (s + 1), :], start=True, stop=True)
                g1a = pA_sb.tile([128, L], BF, tag="g1a")
                if ELU_CUSTOM["g1"]:
                    elu_custom(g1a[:], g1p[:], cs["bg1d"], "g1")
                else:
                    elu_exact(pA_sb, g1a[:], g1p[:], cs["bg1d"], 128)
                g2p = psG.tile([128, L], F32, tag="pg")
                nc.tensor.matmul(g2p[:], cs["g2w"][:], g1a[:], start=True, stop=True)
                sidx = 2 * j + s
                nc.vector.tensor_reduce(
                    gf_all[:, sidx:sidx + 1], g2p[:], axis=mybir.AxisListType.X,
                    op=AX.max)

        # ================= PHASE B: global bias + q/Q projections =============
        nc.vector.tensor_scalar(gf_all[:], gf_all[:], cs["bg2"][:], None, op0=AX.add)
        nc.vector.tensor_copy(gf_bf[:], gf_all[:])
        qp_ = psC.tile([D_LOCAL, B_LOC], F32, tag="pc")
        nc.tensor.matmul(qp_[:], cs["qpwg"][:], gf_bf[:], start=True, stop=False)
        nc.tensor.matmul(qp_[:], cs["qpwp"][:], cprop[:], start=False, stop=True)
        qsb = singles.tile([D_LOCAL, B_LOC], BF, tag="qsb")
        if ELU_CUSTOM["q"]:
            elu_custom(qsb[:], qp_[:], cs["qpb"], "q")
        else:
            elu_exact(pC_sb, qsb[:], qp_[:], cs["qpb"], D_LOCAL)
        Qp = psC.tile([128, B_LOC], F32, tag="pc")
        nc.tensor.matmul(Qp[0:64, :], cs["wq"][:], qsb[:], start=True, stop=True)
        nc.tensor.matmul(Qp[64:128, :], cs["wq"][:], qsb[:], start=True, stop=True,
                         tile_position=(0, 64))
        nc.vector.tensor_scalar(Q_sb[:], Qp[:], cs["bq2"][:], None, op0=AX.add)

        # ================= PHASE C: attention per pair ========================
        for j in range(NPAIR):
            pwj = pw_tiles[j]
            vp = psC.tile([128, L], F32, tag="pc")
            nc.tensor.matmul(vp[0:64, :], cs["wv2"][0:64, :], pwj[0:64, :],
                             start=True, stop=True)
            nc.tensor.matmul(vp[64:128, :], cs["wv2"][64:128, :], pwj[64:128, :],
                             start=True, stop=True, tile_position=(64, 64))
            kp = psC.tile([128, L], F32, tag="pc")
            nc.tensor.matmul(kp[0:64, :], cs["wk2"][0:64, :], pwj[0:64, :],
                             start=True, stop=True)
            nc.tensor.matmul(kp[64:128, :], cs["wk2"][64:128, :], pwj[64:128, :],
                             start=True, stop=True, tile_position=(64, 64))
            vsb = pC_sb.tile([128, L], BF, tag="vsb")
            nc.scalar.activation(vsb[:], vp[:], AF.Identity, bias=cs["bvd"][:])
            ksb = pC_sb.tile([128, L], BF, tag="ksb")
            nc.scalar.activation(ksb[:], kp[:], AF.Identity, bias=cs["bkd"][:])

            sq = pC_sb.tile([128, 64], BF, tag="sq")
            nc.vector.tensor_scalar(sq[0:64, :], cs["csm2"][0:64, :],
                                    Q_sb[0:64, 2 * j:2 * j + 1], None, op0=AX.mult)
            nc.vector.tensor_scalar(sq[64:128, :], cs["csm2"][64:128, :],
                                    Q_sb[64:128, 2 * j + 1:2 * j + 2], None,
                                    op0=AX.mult)
            sp = psC.tile([128, L], F32, tag="pc")
            nc.tensor.matmul(sp[0:64, :], sq[0:64, :], ksb[0:64, :],
                             start=True, stop=True)
            nc.tensor.matmul(sp[64:128, :], sq[64:128, :], ksb[64:128, :],
                             start=True, stop=True, tile_position=(64, 64))
            esb = pC_sb.tile([128, L], BF, tag="esb")
            sume = pC_sb.tile([128, 1], F32, tag="sume")
            nc.scalar.activation(esb[:], sp[:], AF.Exp, accum_out=sume[:])
            rec = pC_sb.tile([128, 1], F32, tag="rec")
            nc.vector.reciprocal(rec[:], sume[:])
            wvt = pC_sb.tile([128, L], BF, tag="wvt")
            nc.gpsimd.tensor_tensor(wvt[:], esb[:], vsb[:], op=AX.mult)
            cu = pC_sb.tile([128, 1], F32, tag="cu")
            nc.vector.tensor_reduce(cu[:], wvt[:], axis=mybir.AxisListType.X, op=AX.add)
            nc.vector.tensor_scalar(ctx_all[:, j:j + 1], cu[:], rec[:], None,
                                    op0=AX.mult)

        # ================= PHASE D: output projection + stores ================
        wlp = psC.tile([128, NPAIR], F32, tag="pc")
        nc.tensor.matmul(wlp[:], cs["wobd"][:], ctx_all[:], start=True, stop=True)
        wl = singles.tile([128, NPAIR], F32, tag="wl")
        nc.vector.tensor_scalar(wl[:], wlp[:], cs["bod"][:], None, op0=AX.add)

        OD = D_LOCAL + D_GLOBAL  # 192
        # even samples' weighted_local: out[2j, f] = wl[f, j]
        nc.sync.dma_start(
            out=dram_ap(out_h, 0, [[1, 64], [2 * OD, NPAIR]]), in_=wl[0:64, :])
        # odd samples
        nc.sync.dma_start(
            out=dram_ap(out_h, OD, [[1, 64], [2 * OD, NPAIR]]), in_=wl[64:128, :])
        # global feat: out[s, 64+g] = gf_all[g, s]
        nc.sync.dma_start(
            out=dram_ap(out_h, 64, [[1, D_GLOBAL], [OD, B_LOC]]), in_=gf_all[:])

    _split_multiwait(nc)
    return nc


# ----------------------------------------------------------------------------
# entry point
# ----------------------------------------------------------------------------
_CACHE = {}


def kernel(**inputs):
    shared = host_prep_shared(inputs)
    cores = host_prep_percore(inputs)

    if "nc" not in _CACHE:
        _CACHE["nc"] = build_nc(shared)
    nc = _CACHE["nc"]

    in_maps = []
    for ci in range(N_CORES):
        m = dict(cores[ci])
        for name, arr in shared.items():
            m[name] = arr
        in_maps.append(m)

    trace = bool(int(os.environ.get("AME2_TRACE", "0")))
    res = run_bass_kernel_spmd(nc, in_maps, core_ids=list(range(N_CORES)),
                               trace=trace)
    if trace and res.exec_time_ns is not None:
        _CACHE["exec_time_ns"] = res.exec_time_ns
    outs = [res.results[ci]["out"] for ci in range(N_CORES)]
    return np.concatenate(outs, axis=0).astype(np.float32)


# revision 13
# speedup vs baseline: 3.8730x; 3.8730x over previous
"""Trainium2 Bass kernel for nn_AME2Encoder (dense_mlp, 8-core data parallel).

Strategy:
  - Pure data parallel: B=2048 sharded 256/core; each core processes its
    samples as 128 "pairs" (2 samples packed on the 128 SBUF partitions),
    with most elementwise work done on 2-pair-wide [128, 1008] tiles to
    amortize per-instruction overheads.
  - Feature-major bf16 activations ([feat, token] tiles, token tile = 504
    = one sample's full 14x36 grid). No transposes needed anywhere.
  - 64-feature layers are packed 2-samples-per-matmul with block-diagonal
    weights (M=128).
  - conv1 (3x3) is a single K=54 matmul per pair over a host-prepared
    im2col layout (input layout prep; all FLOPs stay on device).
  - ELU in 3 passes via the "+1 fold": every ELU site computes
    elu(x)+1 = min(exp(x+b),1) + relu(x+b); the -1 is folded into the next
    layer's bias on the host (scores are softmax-shift-invariant for K,
    and V/global-max shifts fold into constants).
      pass1: ACT Exp(psum + bias) -> e (bf16)
      pass2: ACT Relu(psum + bias) or DVE tensor_scalar -> r (bf16)
      pass3: scalar_tensor_tensor (e min 1) add r -> out (one fused pass)
  - Attention: block-diag 0.5*Q masks -> one scores matmul per pair; ACT
    exp with free sum accumulation; fused V-weighting + context reduction
    in one scalar_tensor_tensor with accum_out.
  - K/V biases: bk drops out of softmax; bv folded into the context
    normalization. So K|V evacuation is a single wide pure-copy pass.
"""

import os
from contextlib import ExitStack

import numpy as np
import ml_dtypes

import concourse.bass as bass
import concourse.mybir as mybir
import concourse.tile as tile
from concourse.bass_utils import run_bass_kernel_spmd
from concourse.vector_clock import ScopedClock


# --- workaround: this walrus rejects the tail Drain carrying >1 sem waits ---
def _patched_dab(self, tick_clock, wait_clock):
    nc = self.nc
    probe = nc.sync.drain()
    wait_clock.add_sem_waits(probe.ins, ScopedClock({None: tick_clock.global_clock}))
    si = probe.ins.sync_info
    waits = list(si.on_wait) if si is not None else []
    if si is not None and len(waits) > 1:
        si.on_wait = waits[:1]
        for w in waits[1:]:
            n2 = nc.sync.drain()
            n2.ins.sync_info = mybir.SyncInfo(on_wait=[w], on_update=[])
    nc.all_engine_barrier()
    assert self.sems is not None
    popped = nc._tile_sem_poison_stack.pop()
    assert popped is self._sem_poison
    nc.clear_and_free_semaphores(list(self.sems.allocated().values()))
    nc.all_engine_barrier()


tile.TileContext._drain_and_barrier = _patched_dab


def _split_multiwait(nc, max_waits=1):
    """This walrus build cannot encode >1 sem-wait on one instruction for some
    structs; hoist excess waits onto EventSemaphore carriers inserted before."""
    ctr = [0]
    for fn in nc.m.functions:
        for blk in fn.blocks:
            insts = list(blk.instructions)
            new = []
            changed = False
            for inst in insts:
                si = inst.sync_info
                waits = list(si.on_wait) if si is not None and si.on_wait else []
                if len(waits) > max_waits:
                    changed = True
                    for w in waits[max_waits:]:
                        ctr[0] += 1
                        new.append(mybir.InstEventSemaphore(
                            name=f"zz_mw_{ctr[0]}", engine=inst.engine,
                            ins=[], outs=[],
                            sync_info=mybir.SyncInfo(on_wait=[w], on_update=[]),
                        ))
                    inst.sync_info = mybir.SyncInfo(
                        on_wait=waits[:max_waits],
                        on_update=list(si.on_update) if si.on_update else [],
                    )
                new.append(inst)
            if changed:
                blk.instructions = new


# ----- problem constants (hardcoded per spec) -----
B, C_IN, H, W = 2048, 3, 14, 36
D_LOCAL, D_POS, D_GLOBAL, D_PROP, NH = 64, 64, 128, 128, 16
HD = D_LOCAL // NH
N_CORES = 8
B_LOC = B // N_CORES      # 256
NPAIR = B_LOC // 2        # 128
NQUAD = NPAIR // 2        # 64 wide iterations (2 pairs each)
L = H * W                 # 504
LP = 512                  # PSUM-bank-aligned half stride
LW = LP + L               # 1016: wide tile width (second half at [LP, LP+L))

BF = mybir.dt.float16
F32 = mybir.dt.float32
bf16 = np.float16
AX = mybir.AluOpType
AF = mybir.ActivationFunctionType

# engine assignment for the flexible passes ("act" or "dve"); stt passes may
# also go to "gp" (gpsimd).
ASSIGN = {
    "r_conv1": "act", "r_conv2": "act", "r_fuse": "act", "r_g1": "dve",
    "kvevac": "dve",
    "stt_conv1": "dve", "stt_conv2": "dve", "stt_fuse": "dve", "stt_g1": "dve",
}


def _np_elu(x):
    return np.where(x > 0, x, np.expm1(np.minimum(x, 0.0)))


# ----------------------------------------------------------------------------
# Host-side constant packing (weight folding / layout prep)
# ----------------------------------------------------------------------------

def _block_diag2(w):
    k, m = w.shape
    out = np.zeros((2 * k, 2 * m), np.float32)
    out[:k, :m] = w
    out[k:, m:] = w
    return out


def _dup_col(b):
    return np.concatenate([b, b]).astype(np.float32)[:, None]


def host_prep_shared(inp):
    """Weight-derived dram parameters. All ELU sites produce elu(x)+1; the -1
    is folded into each consumer's bias here (colsum of the consumer weight)."""
    c = {}
    w1p = inp["conv1_w"].transpose(2, 3, 1, 0).reshape(27, 64)  # k=(3dy+dx)*3+c
    c["w1bd"] = _block_diag2(w1p).astype(bf16)            # [54,128]
    c["b1d"] = _dup_col(inp["conv1_b"])                   # [128,1] f32

    w2 = inp["conv2_w"][:, :, 0, 0].T                     # [in,out]
    c["w2bd"] = _block_diag2(w2).astype(bf16)             # [128,128]
    c["b2d"] = _dup_col(inp["conv2_b"] - w2.sum(axis=0))  # -1 fold (conv1 out)

    fl = inp["fuse_w"][:D_LOCAL]                          # [64,64]
    fp = inp["fuse_w"][D_LOCAL:]                          # [64,64]
    c["wflbd"] = _block_diag2(fl).astype(bf16)            # [128,128]
    c["wfp2"] = np.concatenate([fp, fp], axis=1).astype(bf16)  # [64,128]
    c["bfd"] = _dup_col(inp["fuse_b"] - fl.sum(axis=0))   # -1 fold (conv2 out)

    ys = np.linspace(-1.0, 1.0, H, dtype=np.float32)
    xs = np.linspace(-1.0, 1.0, W, dtype=np.float32)
    gy, gx = np.meshgrid(ys, xs, indexing="ij")
    coords = np.stack([gx, gy], axis=-1).reshape(L, 2)
    pe = _np_elu(coords @ inp["pe_w1"] + inp["pe_b1"]) @ inp["pe_w2"] + inp["pe_b2"]
    c["pec"] = np.ascontiguousarray(pe.T).astype(bf16)    # [64,504] exact

    g1 = inp["g_w1"]                                      # [64,128]
    c["g1w2"] = np.vstack([g1, g1]).astype(bf16)          # [128,128] dup rows
    c["bg1d"] = (inp["g_b1"] - g1.sum(axis=0)).astype(np.float32)[:, None]

    g2 = inp["g_w2"]
    c["g2w"] = g2.astype(bf16)                            # [128,128]
    # applied post-gmax: g2 bias + the -1 fold of g1's elu+1
    c["bg2"] = (inp["g_b2"] - g2.sum(axis=0)).astype(np.float32)[:, None]

    c["wvbd"] = _block_diag2(inp["wv"]).astype(bf16)      # [128,128]
    c["wkbd"] = _block_diag2(inp["wk"]).astype(bf16)      # bk drops in softmax
    # ctx const: bv - colsum(wv) (the elu+1 fold of pw through wv)
    c["cvd"] = _dup_col(inp["bv"] - inp["wv"].sum(axis=0))

    sm = np.zeros((64, 64), np.float32)
    for k in range(64):
        sm[k, (k // HD) * HD:(k // HD + 1) * HD] = 1.0 / np.sqrt(HD)
    c["csmbd"] = _block_diag2(sm).astype(bf16)            # [128,128]

    c["qpwg"] = inp["qp_w"][:D_GLOBAL].astype(bf16)       # [128,64]
    c["qpwp"] = inp["qp_w"][D_GLOBAL:].astype(bf16)       # [128,64]
    c["qpb"] = inp["qp_b"].astype(np.float32)[:, None]    # [64,1]

    wq = inp["wq"]
    c["wq"] = wq.astype(bf16)
    # q is produced as elu+1; fold the -1 into bq (exact, linear)
    c["bq2"] = _dup_col(inp["bq"] - wq.sum(axis=0))       # [128,1]

    c["wobd"] = _block_diag2(inp["wo"]).astype(bf16)      # [128,128]
    c["bod"] = _dup_col(inp["bo"])
    return c


def host_prep_percore(inp):
    """Per-core data params: im2col input layout + transposed prop_emb."""
    mf = inp["map_feat"].astype(np.float32)
    mp = np.zeros((B, 3, H + 2, W + 2), np.float32)
    mp[:, :, 1:H + 1, 1:W + 1] = mf
    from numpy.lib.stride_tricks import sliding_window_view
    sw = sliding_window_view(mp, (3, 3), axis=(2, 3))     # [B,3,14,36,3,3]
    ic = sw.transpose(0, 4, 5, 1, 2, 3).reshape(B, 27, L)  # k=(3dy+dx)*3+c
    ic = np.ascontiguousarray(ic).reshape(B // 2, 54, L).astype(bf16)
    prop = inp["prop_emb"].astype(np.float32)
    cores = []
    for ci in range(N_CORES):
        sl = slice(ci * B_LOC, (ci + 1) * B_LOC)
        cores.append({
            "ic": np.ascontiguousarray(ic[ci * NPAIR:(ci + 1) * NPAIR]),
            "propT": np.ascontiguousarray(prop[sl].T).astype(bf16),  # [128,256]
        })
    return cores


# ----------------------------------------------------------------------------
# Bass graph
# ----------------------------------------------------------------------------

def build_nc(shared):
    nc = bass.Bass()

    P = {}
    P["ic"] = nc.declare_dram_parameter("ic", [NPAIR, 54, L], BF, isOutput=False)
    P["propT"] = nc.declare_dram_parameter("propT", [D_PROP, B_LOC], BF,
                                           isOutput=False)
    for name, arr in shared.items():
        dt = BF if arr.dtype == bf16 else F32
        P[name] = nc.declare_dram_parameter(name, list(arr.shape), dt,
                                            isOutput=False)
    out_h = nc.declare_dram_parameter("out", [B_LOC, D_LOCAL + D_GLOBAL], F32,
                                      isOutput=True)

    def dram_ap(h, offset, dims):
        base = h[:]
        return bass.AP(tensor=base.tensor, offset=offset,
                       ap=[list(d) for d in dims])

    with tile.TileContext(nc) as tc, ExitStack() as ctx:
        singles = ctx.enter_context(tc.tile_pool(name="singles", bufs=1))
        pA_in = ctx.enter_context(tc.tile_pool(name="pA_in", bufs=4))
        pA_sb = ctx.enter_context(tc.tile_pool(name="pA_sb", bufs=2))
        pC_sb = ctx.enter_context(tc.tile_pool(name="pC_sb", bufs=2))
        psA = ctx.enter_context(tc.tile_pool(name="psA", bufs=2, space="PSUM"))
        psG = ctx.enter_context(tc.tile_pool(name="psG", bufs=2, space="PSUM"))

        # ---- constants ----
        cs = {}
        for name, arr in shared.items():
            dt = BF if arr.dtype == bf16 else F32
            t = singles.tile(list(arr.shape), dt, tag=f"c_{name}",
                             name=f"c_{name}")
            nc.sync.dma_start(out=t[:], in_=P[name][:])
            cs[name] = t
        cprop = singles.tile([D_PROP, B_LOC], BF, tag="c_prop", name="c_prop")
        nc.sync.dma_start(out=cprop[:], in_=P["propT"][:])

        # persistent state
        gf_all = singles.tile([D_GLOBAL, B_LOC], F32, tag="gf_all", name="gf_all")
        gf_bf = singles.tile([D_GLOBAL, B_LOC], BF, tag="gf_bf", name="gf_bf")
        ctx_all = singles.tile([128, NPAIR], BF, tag="ctx_all", name="ctx_all")
        Q2 = singles.tile([128, NPAIR], F32, tag="Q2", name="Q2")
        pw_tiles = [singles.tile([128, LW], BF, tag=f"pw{q}", name=f"pw{q}")
                    for q in range(NQUAD)]

        ENG = {"act": nc.scalar, "dve": nc.vector, "gp": nc.gpsimd}

        def elu1(pool, dst_ap, src_ap, bias_tile, site, nparts=128):
            """dst = elu(src + b) + 1 = min(exp(src+b),1) + relu(src+b)."""
            n = src_ap.shape[-1]
            e = pool.tile([nparts, n], BF, tag="elu_e", name="elu_e")
            r = pool.tile([nparts, n], BF, tag="elu_r", name="elu_r")
            nc.scalar.activation(e[:], src_ap, AF.Exp, bias=bias_tile[:],
                                 scale=1.0)
            if ASSIGN[f"r_{site}"] == "act":
                nc.scalar.activation(r[:], src_ap, AF.Relu, bias=bias_tile[:],
                                     scale=1.0)
            else:
                nc.vector.tensor_scalar(r[:], src_ap, bias_tile[:], 0.0,
                                        op0=AX.add, op1=AX.max)
            ENG[ASSIGN[f"stt_{site}"]].scalar_tensor_tensor(
                dst_ap, e[:], 1.0, r[:], op0=AX.min, op1=AX.add)

        # ================= PHASE A: conv/fuse/global, 2 pairs per iter =======
        # wide tiles hold two 504-token halves at bank-aligned offsets 0, LP
        H0 = slice(0, L)
        H1 = slice(LP, LP + L)
        HS = (H0, H1)
        for q in range(NQUAD):
            ict = pA_in.tile([54, LW], BF, tag="ict", name="ict")
            nc.sync.dma_start(out=ict[:, H0], in_=P["ic"][2 * q])
            nc.sync.dma_start(out=ict[:, H1], in_=P["ic"][2 * q + 1])

            c1p = psA.tile([128, LW], F32, tag="pa", name="c1p")
            for h in (0, 1):
                nc.tensor.matmul(c1p[:, HS[h]], cs["w1bd"][:], ict[:, HS[h]],
                                 start=True, stop=True)
            a1 = pA_sb.tile([128, LW], BF, tag="a1", name="a1")
            elu1(pA_sb, a1[:], c1p[:], cs["b1d"], "conv1")

            c2p = psA.tile([128, LW], F32, tag="pa", name="c2p")
            for h in (0, 1):
                nc.tensor.matmul(c2p[:, HS[h]], cs["w2bd"][:], a1[:, HS[h]],
                                 start=True, stop=True)
            a2 = pA_sb.tile([128, LW], BF, tag="a2", name="a2")
            elu1(pA_sb, a2[:], c2p[:], cs["b2d"], "conv2")

            fp_ = psA.tile([128, LW], F32, tag="pa", name="fp_")
            for h in (0, 1):
                nc.tensor.matmul(fp_[:, HS[h]], cs["wflbd"][:], a2[:, HS[h]],
                                 start=True, stop=False)
                nc.tensor.matmul(fp_[:, HS[h]], cs["wfp2"][:], cs["pec"][:],
                                 start=False, stop=True)
            pwq = pw_tiles[q]
            elu1(pA_sb, pwq[:], fp_[:], cs["bfd"], "fuse")

            # global branch per pair (samples A/B side by side on free dim)
            for h in (0, 1):
                j = 2 * q + h
                psl = HS[h]
                g1p = psG.tile([128, LW], F32, tag="pg", name="g1p")
                nc.tensor.matmul(g1p[:, H0], cs["g1w2"][0:64, :],
                                 pwq[0:64, psl], start=True, stop=True)
                nc.tensor.matmul(g1p[:, H1], cs["g1w2"][64:128, :],
                                 pwq[64:128, psl], start=True, stop=True)
                g1a = pA_sb.tile([128, LW], BF, tag="g1a", name="g1a")
                elu1(pA_sb, g1a[:], g1p[:], cs["bg1d"], "g1")
                g2p = psG.tile([128, LW], F32, tag="pg", name="g2p")
                for s in (0, 1):
                    nc.tensor.matmul(g2p[:, HS[s]], cs["g2w"][:],
                                     g1a[:, HS[s]], start=True, stop=True)
                for s in (0, 1):
                    sidx = 2 * j + s
                    nc.vector.tensor_reduce(
                        gf_all[:, sidx:sidx + 1], g2p[:, HS[s]],
                        axis=mybir.AxisListType.X, op=AX.max)

        # ================= PHASE B: global bias + q/Q projections =============
        nc.vector.tensor_scalar(gf_all[:], gf_all[:], cs["bg2"][:], None,
                                op0=AX.add)
        nc.vector.tensor_copy(gf_bf[:], gf_all[:])
        qp_ = psG.tile([D_LOCAL, B_LOC], F32, tag="pg", name="qp_")
        nc.tensor.matmul(qp_[:], cs["qpwg"][:], gf_bf[:], start=True, stop=False)
        nc.tensor.matmul(qp_[:], cs["qpwp"][:], cprop[:], start=False, stop=True)
        # q = elu(qp + qp_b) + 1   (the -1 is folded into bq2)
        qe = singles.tile([D_LOCAL, B_LOC], BF, tag="qe", name="qe")
        qr = singles.tile([D_LOCAL, B_LOC], BF, tag="qr", name="qr")
        qsb = singles.tile([D_LOCAL, B_LOC], BF, tag="qsb", name="qsb")
        nc.scalar.activation(qe[:], qp_[:], AF.Exp, bias=cs["qpb"][:], scale=1.0)
        nc.scalar.activation(qr[:], qp_[:], AF.Relu, bias=cs["qpb"][:], scale=1.0)
        nc.vector.scalar_tensor_tensor(qsb[:], qe[:], 1.0, qr[:],
                                       op0=AX.min, op1=AX.add)
        # Q2 [128, NPAIR]: rows 0:64 = Q[:, even samples], 64:128 = odd
        Qp = psG.tile([128, NPAIR], F32, tag="pg", name="Qp")
        qs_even = qsb[:].rearrange("p (j s) -> p s j", s=2)
        nc.tensor.matmul(Qp[0:64, :], cs["wq"][:], qs_even[:, 0, :],
                         start=True, stop=True)
        nc.tensor.matmul(Qp[64:128, :], cs["wq"][:], qs_even[:, 1, :],
                         start=True, stop=True, tile_position=(0, 64))
        nc.vector.tensor_scalar(Q2[:], Qp[:], cs["bq2"][:], None, op0=AX.add)

        # ================= PHASE C: attention per pair ========================
        for j in range(NPAIR):
            pwj = pw_tiles[j // 2][:, (j % 2) * LP:(j % 2) * LP + L]
            vkp = psA.tile([128, LW], F32, tag="pa", name="vkp")
            nc.tensor.matmul(vkp[:, 0:L], cs["wvbd"][:], pwj, start=True,
                             stop=True)
            nc.tensor.matmul(vkp[:, LP:LP + L], cs["wkbd"][:], pwj, start=True,
                             stop=True)
            vks = pC_sb.tile([128, LW], BF, tag="vks", name="vks")
            if ASSIGN["kvevac"] == "act":
                nc.scalar.activation(vks[:], vkp[:], AF.Copy)
            else:
                nc.vector.tensor_copy(vks[:], vkp[:])
            sqbd = pC_sb.tile([128, 128], BF, tag="sqbd", name="sqbd")
            nc.vector.tensor_scalar(sqbd[:], cs["csmbd"][:], Q2[:, j:j + 1],
                                    None, op0=AX.mult)
            scp = psG.tile([128, LW], F32, tag="pg", name="scp")
            nc.tensor.matmul(scp[:, 0:L], sqbd[:], vks[:, LP:LP + L],
                             start=True, stop=True)
            esb = pC_sb.tile([128, L], BF, tag="esb", name="esb")
            sume = pC_sb.tile([128, 1], F32, tag="sume", name="sume")
            nc.scalar.activation(esb[:], scp[:, 0:L], AF.Exp,
                                 accum_out=sume[:])
            rec = pC_sb.tile([128, 1], F32, tag="rec", name="rec")
            nc.vector.reciprocal(rec[:], sume[:])
            wvt = pC_sb.tile([128, L], BF, tag="wvt", name="wvt")
            ctxu = pC_sb.tile([128, 1], F32, tag="ctxu", name="ctxu")
            nc.vector.scalar_tensor_tensor(wvt[:], esb[:], 1.0, vks[:, 0:L],
                                           op0=AX.mult, op1=AX.mult,
                                           accum_out=ctxu[:])
            nc.vector.tensor_scalar(ctx_all[:, j:j + 1], ctxu[:], rec[:],
                                    cs["cvd"][:], op0=AX.mult, op1=AX.add)

        # ================= PHASE D: output projection + stores ================
        wlp = psG.tile([128, NPAIR], F32, tag="pg", name="wlp")
        nc.tensor.matmul(wlp[:], cs["wobd"][:], ctx_all[:], start=True,
                         stop=True)
        wl = singles.tile([128, NPAIR], F32, tag="wl", name="wl")
        nc.vector.tensor_scalar(wl[:], wlp[:], cs["bod"][:], None, op0=AX.add)

        OD = D_LOCAL + D_GLOBAL  # 192
        nc.sync.dma_start(
            out=dram_ap(out_h, 0, [[1, 64], [2 * OD, NPAIR]]), in_=wl[0:64, :])
        nc.sync.dma_start(
            out=dram_ap(out_h, OD, [[1, 64], [2 * OD, NPAIR]]),
            in_=wl[64:128, :])
        nc.sync.dma_start(
            out=dram_ap(out_h, 64, [[1, D_GLOBAL], [OD, B_LOC]]), in_=gf_all[:])

    _split_multiwait(nc)
    return nc


# ----------------------------------------------------------------------------
# entry point
# ----------------------------------------------------------------------------
_CACHE = {}


def kernel(**inputs):
    shared = host_prep_shared(inputs)
    cores = host_prep_percore(inputs)

    if "nc" not in _CACHE:
        _CACHE["nc"] = build_nc(shared)
    nc = _CACHE["nc"]

    in_maps = []
    for ci in range(N_CORES):
        m = dict(cores[ci])
        for name, arr in shared.items():
            m[name] = arr
        in_maps.append(m)

    trace = bool(int(os.environ.get("AME2_TRACE", "0")))
    res = run_bass_kernel_spmd(nc, in_maps, core_ids=list(range(N_CORES)),
                               trace=trace)
    if trace and res.exec_time_ns is not None:
        _CACHE["exec_time_ns"] = res.exec_time_ns
    outs = [res.results[ci]["out"] for ci in range(N_CORES)]
    return np.concatenate(outs, axis=0).astype(np.float32)


# revision 14
# speedup vs baseline: 4.2166x; 1.0887x over previous
"""Trainium2 Bass kernel for nn_AME2Encoder (dense_mlp, 8-core data parallel).

Strategy:
  - Pure data parallel: B=2048 sharded 256/core; each core processes its
    samples as 128 "pairs" (2 samples packed on the 128 SBUF partitions),
    with most elementwise work done on 2-pair-wide [128, 1008] tiles to
    amortize per-instruction overheads.
  - Feature-major bf16 activations ([feat, token] tiles, token tile = 504
    = one sample's full 14x36 grid). No transposes needed anywhere.
  - 64-feature layers are packed 2-samples-per-matmul with block-diagonal
    weights (M=128).
  - conv1 (3x3) is a single K=54 matmul per pair over a host-prepared
    im2col layout (input layout prep; all FLOPs stay on device).
  - ELU in 3 passes via the "+1 fold": every ELU site computes
    elu(x)+1 = min(exp(x+b),1) + relu(x+b); the -1 is folded into the next
    layer's bias on the host (scores are softmax-shift-invariant for K,
    and V/global-max shifts fold into constants).
      pass1: ACT Exp(psum + bias) -> e (bf16)
      pass2: ACT Relu(psum + bias) or DVE tensor_scalar -> r (bf16)
      pass3: scalar_tensor_tensor (e min 1) add r -> out (one fused pass)
  - Attention: block-diag 0.5*Q masks -> one scores matmul per pair; ACT
    exp with free sum accumulation; fused V-weighting + context reduction
    in one scalar_tensor_tensor with accum_out.
  - K/V biases: bk drops out of softmax; bv folded into the context
    normalization. So K|V evacuation is a single wide pure-copy pass.
"""

import os
from contextlib import ExitStack

import numpy as np
import ml_dtypes

import concourse.bass as bass
import concourse.mybir as mybir
import concourse.tile as tile
from concourse.bass_utils import run_bass_kernel_spmd
from concourse.vector_clock import ScopedClock


# --- workaround: this walrus rejects the tail Drain carrying >1 sem waits ---
def _patched_dab(self, tick_clock, wait_clock):
    nc = self.nc
    probe = nc.sync.drain()
    wait_clock.add_sem_waits(probe.ins, ScopedClock({None: tick_clock.global_clock}))
    si = probe.ins.sync_info
    waits = list(si.on_wait) if si is not None else []
    if si is not None and len(waits) > 1:
        si.on_wait = waits[:1]
        for w in waits[1:]:
            n2 = nc.sync.drain()
            n2.ins.sync_info = mybir.SyncInfo(on_wait=[w], on_update=[])
    nc.all_engine_barrier()
    assert self.sems is not None
    popped = nc._tile_sem_poison_stack.pop()
    assert popped is self._sem_poison
    nc.clear_and_free_semaphores(list(self.sems.allocated().values()))
    nc.all_engine_barrier()


tile.TileContext._drain_and_barrier = _patched_dab


def _split_multiwait(nc, max_waits=1):
    """This walrus build cannot encode >1 sem-wait on one instruction for some
    structs; hoist excess waits onto EventSemaphore carriers inserted before."""
    ctr = [0]
    for fn in nc.m.functions:
        for blk in fn.blocks:
            insts = list(blk.instructions)
            new = []
            changed = False
            for inst in insts:
                si = inst.sync_info
                waits = list(si.on_wait) if si is not None and si.on_wait else []
                if len(waits) > max_waits:
                    changed = True
                    for w in waits[max_waits:]:
                        ctr[0] += 1
                        new.append(mybir.InstEventSemaphore(
                            name=f"zz_mw_{ctr[0]}", engine=inst.engine,
                            ins=[], outs=[],
                            sync_info=mybir.SyncInfo(on_wait=[w], on_update=[]),
                        ))
                    inst.sync_info = mybir.SyncInfo(
                        on_wait=waits[:max_waits],
                        on_update=list(si.on_update) if si.on_update else [],
                    )
                new.append(inst)
            if changed:
                blk.instructions = new


# ----- problem constants (hardcoded per spec) -----
B, C_IN, H, W = 2048, 3, 14, 36
D_LOCAL, D_POS, D_GLOBAL, D_PROP, NH = 64, 64, 128, 128, 16
HD = D_LOCAL // NH
N_CORES = 8
B_LOC = B // N_CORES      # 256
NPAIR = B_LOC // 2        # 128
NQUAD = NPAIR // 2        # 64 wide iterations (2 pairs each)
L = H * W                 # 504
LP = 512                  # PSUM-bank-aligned half stride
LW = LP + L               # 1016: wide tile width (second half at [LP, LP+L))

BF = mybir.dt.float16
F32 = mybir.dt.float32
bf16 = np.float16
AX = mybir.AluOpType
AF = mybir.ActivationFunctionType

# engine assignment for the flexible passes ("act" or "dve"); stt passes may
# also go to "gp" (gpsimd).
ASSIGN = {
    "r_conv1": "act", "r_conv2": "act", "r_fuse": "act", "r_g1": "dve",
    "kvevac": "act",
    "stt_conv1": "dve", "stt_conv2": "dve", "stt_fuse": "dve", "stt_g1": "dve",
}


def _np_elu(x):
    return np.where(x > 0, x, np.expm1(np.minimum(x, 0.0)))


# ----------------------------------------------------------------------------
# Host-side constant packing (weight folding / layout prep)
# ----------------------------------------------------------------------------

def _block_diag2(w):
    k, m = w.shape
    out = np.zeros((2 * k, 2 * m), np.float32)
    out[:k, :m] = w
    out[k:, m:] = w
    return out


def _dup_col(b):
    return np.concatenate([b, b]).astype(np.float32)[:, None]


def host_prep_shared(inp):
    """Weight-derived dram parameters. All ELU sites produce elu(x)+1; the -1
    is folded into each consumer's bias here (colsum of the consumer weight)."""
    c = {}
    w1p = inp["conv1_w"].transpose(2, 3, 1, 0).reshape(27, 64)  # k=(3dy+dx)*3+c
    c["w1bd"] = _block_diag2(w1p).astype(bf16)            # [54,128]
    c["b1d"] = _dup_col(inp["conv1_b"])                   # [128,1] f32

    w2 = inp["conv2_w"][:, :, 0, 0].T                     # [in,out]
    c["w2bd"] = _block_diag2(w2).astype(bf16)             # [128,128]
    c["b2d"] = _dup_col(inp["conv2_b"])

    fl = inp["fuse_w"][:D_LOCAL]                          # [64,64]
    fp = inp["fuse_w"][D_LOCAL:]                          # [64,64]
    c["wflbd"] = _block_diag2(fl).astype(bf16)            # [128,128]
    c["wfp2"] = np.concatenate([fp, fp], axis=1).astype(bf16)  # [64,128]
    c["bfd"] = _dup_col(inp["fuse_b"])

    ys = np.linspace(-1.0, 1.0, H, dtype=np.float32)
    xs = np.linspace(-1.0, 1.0, W, dtype=np.float32)
    gy, gx = np.meshgrid(ys, xs, indexing="ij")
    coords = np.stack([gx, gy], axis=-1).reshape(L, 2)
    pe = _np_elu(coords @ inp["pe_w1"] + inp["pe_b1"]) @ inp["pe_w2"] + inp["pe_b2"]
    c["pec"] = np.ascontiguousarray(pe.T).astype(bf16)    # [64,504] exact

    g1 = inp["g_w1"]                                      # [64,128]
    c["g1w2"] = np.vstack([g1, g1]).astype(bf16)          # [128,128] dup rows
    c["bg1d"] = inp["g_b1"].astype(np.float32)[:, None]

    g2 = inp["g_w2"]
    c["g2w"] = g2.astype(bf16)                            # [128,128]
    c["bg2"] = inp["g_b2"].astype(np.float32)[:, None]    # applied post-gmax

    c["wvbd"] = _block_diag2(inp["wv"]).astype(bf16)      # [128,128]
    c["wkbd"] = _block_diag2(inp["wk"]).astype(bf16)      # bk drops in softmax
    c["cvd"] = _dup_col(inp["bv"])  # folded into ctx normalization

    sm = np.zeros((64, 64), np.float32)
    for k in range(64):
        sm[k, (k // HD) * HD:(k // HD + 1) * HD] = 1.0 / np.sqrt(HD)
    c["csmbd"] = _block_diag2(sm).astype(bf16)            # [128,128]

    c["qpwg"] = inp["qp_w"][:D_GLOBAL].astype(bf16)       # [128,64]
    c["qpwp"] = inp["qp_w"][D_GLOBAL:].astype(bf16)       # [128,64]
    c["qpb"] = inp["qp_b"].astype(np.float32)[:, None]    # [64,1]

    c["wq"] = inp["wq"].astype(bf16)
    c["bq2"] = _dup_col(inp["bq"])                        # [128,1]

    c["wobd"] = _block_diag2(inp["wo"]).astype(bf16)      # [128,128]
    c["bod"] = _dup_col(inp["bo"])
    return c


def host_prep_percore(inp):
    """Per-core data params: im2col input layout + transposed prop_emb."""
    mf = inp["map_feat"].astype(np.float32)
    mp = np.zeros((B, 3, H + 2, W + 2), np.float32)
    mp[:, :, 1:H + 1, 1:W + 1] = mf
    from numpy.lib.stride_tricks import sliding_window_view
    sw = sliding_window_view(mp, (3, 3), axis=(2, 3))     # [B,3,14,36,3,3]
    ic = sw.transpose(0, 4, 5, 1, 2, 3).reshape(B, 27, L)  # k=(3dy+dx)*3+c
    ic = np.ascontiguousarray(ic).reshape(B // 2, 54, L).astype(bf16)
    prop = inp["prop_emb"].astype(np.float32)
    cores = []
    for ci in range(N_CORES):
        sl = slice(ci * B_LOC, (ci + 1) * B_LOC)
        cores.append({
            "ic": np.ascontiguousarray(ic[ci * NPAIR:(ci + 1) * NPAIR]),
            "propT": np.ascontiguousarray(prop[sl].T).astype(bf16),  # [128,256]
        })
    return cores


# ----------------------------------------------------------------------------
# Bass graph
# ----------------------------------------------------------------------------

def build_nc(shared):
    nc = bass.Bass()

    P = {}
    P["ic"] = nc.declare_dram_parameter("ic", [NPAIR, 54, L], BF, isOutput=False)
    P["propT"] = nc.declare_dram_parameter("propT", [D_PROP, B_LOC], BF,
                                           isOutput=False)
    for name, arr in shared.items():
        dt = BF if arr.dtype == bf16 else F32
        P[name] = nc.declare_dram_parameter(name, list(arr.shape), dt,
                                            isOutput=False)
    out_h = nc.declare_dram_parameter("out", [B_LOC, D_LOCAL + D_GLOBAL], F32,
                                      isOutput=True)

    def dram_ap(h, offset, dims):
        base = h[:]
        return bass.AP(tensor=base.tensor, offset=offset,
                       ap=[list(d) for d in dims])

    with tile.TileContext(nc) as tc, ExitStack() as ctx:
        singles = ctx.enter_context(tc.tile_pool(name="singles", bufs=1))
        pA_in = ctx.enter_context(tc.tile_pool(name="pA_in", bufs=4))
        pA_sb = ctx.enter_context(tc.tile_pool(name="pA_sb", bufs=2))
        pC_sb = ctx.enter_context(tc.tile_pool(name="pC_sb", bufs=2))
        psA = ctx.enter_context(tc.tile_pool(name="psA", bufs=2, space="PSUM"))
        psG = ctx.enter_context(tc.tile_pool(name="psG", bufs=2, space="PSUM"))

        # ---- constants ----
        cs = {}
        for name, arr in shared.items():
            dt = BF if arr.dtype == bf16 else F32
            t = singles.tile(list(arr.shape), dt, tag=f"c_{name}",
                             name=f"c_{name}")
            nc.sync.dma_start(out=t[:], in_=P[name][:])
            cs[name] = t
        cprop = singles.tile([D_PROP, B_LOC], BF, tag="c_prop", name="c_prop")
        nc.sync.dma_start(out=cprop[:], in_=P["propT"][:])

        # persistent state
        gf_all = singles.tile([D_GLOBAL, B_LOC], F32, tag="gf_all", name="gf_all")
        gf_bf = singles.tile([D_GLOBAL, B_LOC], BF, tag="gf_bf", name="gf_bf")
        ctx_all = singles.tile([128, NPAIR], BF, tag="ctx_all", name="ctx_all")
        Q2 = singles.tile([128, NPAIR], F32, tag="Q2", name="Q2")
        pw_tiles = [singles.tile([128, LW], BF, tag=f"pw{q}", name=f"pw{q}")
                    for q in range(NQUAD)]

        ENG = {"act": nc.scalar, "dve": nc.vector, "gp": nc.gpsimd}

        def elu1(pool, dst_ap, src_ap, bias_tile, site, nparts=128):
            """dst = elu(src + b) = min(exp(src+b),1) - 1 + relu(src+b)."""
            n = src_ap.shape[-1]
            e = pool.tile([nparts, n], BF, tag="elu_e", name="elu_e")
            r = pool.tile([nparts, n], BF, tag="elu_r", name="elu_r")
            f = pool.tile([nparts, n], BF, tag="elu_f", name="elu_f")
            nc.scalar.activation(e[:], src_ap, AF.Exp, bias=bias_tile[:],
                                 scale=1.0)
            if ASSIGN[f"r_{site}"] == "act":
                nc.scalar.activation(r[:], src_ap, AF.Relu, bias=bias_tile[:],
                                     scale=1.0)
            else:
                nc.vector.tensor_scalar(r[:], src_ap, bias_tile[:], 0.0,
                                        op0=AX.add, op1=AX.max)
            nc.vector.tensor_scalar(f[:], e[:], 1.0, -1.0,
                                    op0=AX.min, op1=AX.add)
            nc.vector.tensor_tensor(dst_ap, f[:], r[:], op=AX.add)

        # ================= PHASE A: conv/fuse/global, 2 pairs per iter =======
        # wide tiles hold two 504-token halves at bank-aligned offsets 0, LP
        H0 = slice(0, L)
        H1 = slice(LP, LP + L)
        HS = (H0, H1)
        for q in range(NQUAD):
            ict = pA_in.tile([54, LW], BF, tag="ict", name="ict")
            nc.sync.dma_start(out=ict[:, H0], in_=P["ic"][2 * q])
            nc.sync.dma_start(out=ict[:, H1], in_=P["ic"][2 * q + 1])

            c1p = psA.tile([128, LW], F32, tag="pa", name="c1p")
            for h in (0, 1):
                nc.tensor.matmul(c1p[:, HS[h]], cs["w1bd"][:], ict[:, HS[h]],
                                 start=True, stop=True)
            a1 = pA_sb.tile([128, LW], BF, tag="a1", name="a1")
            elu1(pA_sb, a1[:], c1p[:], cs["b1d"], "conv1")

            c2p = psA.tile([128, LW], F32, tag="pa", name="c2p")
            for h in (0, 1):
                nc.tensor.matmul(c2p[:, HS[h]], cs["w2bd"][:], a1[:, HS[h]],
                                 start=True, stop=True)
            a2 = pA_sb.tile([128, LW], BF, tag="a2", name="a2")
            elu1(pA_sb, a2[:], c2p[:], cs["b2d"], "conv2")

            fp_ = psA.tile([128, LW], F32, tag="pa", name="fp_")
            for h in (0, 1):
                nc.tensor.matmul(fp_[:, HS[h]], cs["wflbd"][:], a2[:, HS[h]],
                                 start=True, stop=False)
                nc.tensor.matmul(fp_[:, HS[h]], cs["wfp2"][:], cs["pec"][:],
                                 start=False, stop=True)
            pwq = pw_tiles[q]
            elu1(pA_sb, pwq[:], fp_[:], cs["bfd"], "fuse")

            # global branch per pair (samples A/B side by side on free dim)
            for h in (0, 1):
                j = 2 * q + h
                psl = HS[h]
                g1p = psG.tile([128, LW], F32, tag="pg", name="g1p")
                nc.tensor.matmul(g1p[:, H0], cs["g1w2"][0:64, :],
                                 pwq[0:64, psl], start=True, stop=True)
                nc.tensor.matmul(g1p[:, H1], cs["g1w2"][64:128, :],
                                 pwq[64:128, psl], start=True, stop=True)
                g1a = pA_sb.tile([128, LW], BF, tag="g1a", name="g1a")
                elu1(pA_sb, g1a[:], g1p[:], cs["bg1d"], "g1")
                g2p = psG.tile([128, LW], F32, tag="pg", name="g2p")
                for s in (0, 1):
                    nc.tensor.matmul(g2p[:, HS[s]], cs["g2w"][:],
                                     g1a[:, HS[s]], start=True, stop=True)
                for s in (0, 1):
                    sidx = 2 * j + s
                    nc.vector.tensor_reduce(
                        gf_all[:, sidx:sidx + 1], g2p[:, HS[s]],
                        axis=mybir.AxisListType.X, op=AX.max)

        # ================= PHASE B: global bias + q/Q projections =============
        nc.vector.tensor_scalar(gf_all[:], gf_all[:], cs["bg2"][:], None,
                                op0=AX.add)
        nc.vector.tensor_copy(gf_bf[:], gf_all[:])
        qp_ = psG.tile([D_LOCAL, B_LOC], F32, tag="pg", name="qp_")
        nc.tensor.matmul(qp_[:], cs["qpwg"][:], gf_bf[:], start=True, stop=False)
        nc.tensor.matmul(qp_[:], cs["qpwp"][:], cprop[:], start=False, stop=True)
        # q = elu(qp + qp_b)
        qe = singles.tile([D_LOCAL, B_LOC], BF, tag="qe", name="qe")
        qr = singles.tile([D_LOCAL, B_LOC], BF, tag="qr", name="qr")
        qsb = singles.tile([D_LOCAL, B_LOC], BF, tag="qsb", name="qsb")
        nc.scalar.activation(qe[:], qp_[:], AF.Exp, bias=cs["qpb"][:], scale=1.0)
        nc.scalar.activation(qr[:], qp_[:], AF.Relu, bias=cs["qpb"][:], scale=1.0)
        nc.vector.tensor_scalar(qe[:], qe[:], 1.0, -1.0, op0=AX.min, op1=AX.add)
        nc.vector.tensor_tensor(qsb[:], qe[:], qr[:], op=AX.add)
        # Q2 [128, NPAIR]: rows 0:64 = Q[:, even samples], 64:128 = odd
        Qp = psG.tile([128, NPAIR], F32, tag="pg", name="Qp")
        qs_even = qsb[:].rearrange("p (j s) -> p s j", s=2)
        nc.tensor.matmul(Qp[0:64, :], cs["wq"][:], qs_even[:, 0, :],
                         start=True, stop=True)
        nc.tensor.matmul(Qp[64:128, :], cs["wq"][:], qs_even[:, 1, :],
                         start=True, stop=True, tile_position=(0, 64))
        nc.vector.tensor_scalar(Q2[:], Qp[:], cs["bq2"][:], None, op0=AX.add)

        # ================= PHASE C: attention per pair ========================
        for j in range(NPAIR):
            pwj = pw_tiles[j // 2][:, (j % 2) * LP:(j % 2) * LP + L]
            vkp = psA.tile([128, LW], F32, tag="pa", name="vkp")
            nc.tensor.matmul(vkp[:, 0:L], cs["wvbd"][:], pwj, start=True,
                             stop=True)
            nc.tensor.matmul(vkp[:, LP:LP + L], cs["wkbd"][:], pwj, start=True,
                             stop=True)
            vks = pC_sb.tile([128, LW], BF, tag="vks", name="vks")
            if ASSIGN["kvevac"] == "act":
                nc.scalar.activation(vks[:], vkp[:], AF.Copy)
            else:
                nc.vector.tensor_copy(vks[:], vkp[:])
            sqbd = pC_sb.tile([128, 128], BF, tag="sqbd", name="sqbd")
            nc.vector.tensor_scalar(sqbd[:], cs["csmbd"][:], Q2[:, j:j + 1],
                                    None, op0=AX.mult)
            scp = psG.tile([128, LW], F32, tag="pg", name="scp")
            nc.tensor.matmul(scp[:, 0:L], sqbd[:], vks[:, LP:LP + L],
                             start=True, stop=True)
            esb = pC_sb.tile([128, L], BF, tag="esb", name="esb")
            sume = pC_sb.tile([128, 1], F32, tag="sume", name="sume")
            nc.scalar.activation(esb[:], scp[:, 0:L], AF.Exp,
                                 accum_out=sume[:])
            rec = pC_sb.tile([128, 1], F32, tag="rec", name="rec")
            nc.vector.reciprocal(rec[:], sume[:])
            wvt = pC_sb.tile([128, L], BF, tag="wvt", name="wvt")
            ctxu = pC_sb.tile([128, 1], F32, tag="ctxu", name="ctxu")
            nc.vector.scalar_tensor_tensor(wvt[:], esb[:], 1.0, vks[:, 0:L],
                                           op0=AX.mult, op1=AX.mult,
                                           accum_out=ctxu[:])
            nc.vector.tensor_scalar(ctx_all[:, j:j + 1], ctxu[:], rec[:],
                                    cs["cvd"][:], op0=AX.mult, op1=AX.add)

        # ================= PHASE D: output projection + stores ================
        wlp = psG.tile([128, NPAIR], F32, tag="pg", name="wlp")
        nc.tensor.matmul(wlp[:], cs["wobd"][:], ctx_all[:], start=True,
                         stop=True)
        wl = singles.tile([128, NPAIR], F32, tag="wl", name="wl")
        nc.vector.tensor_scalar(wl[:], wlp[:], cs["bod"][:], None, op0=AX.add)

        OD = D_LOCAL + D_GLOBAL  # 192
        nc.sync.dma_start(
            out=dram_ap(out_h, 0, [[1, 64], [2 * OD, NPAIR]]), in_=wl[0:64, :])
        nc.sync.dma_start(
            out=dram_ap(out_h, OD, [[1, 64], [2 * OD, NPAIR]]),
            in_=wl[64:128, :])
        nc.sync.dma_start(
            out=dram_ap(out_h, 64, [[1, D_GLOBAL], [OD, B_LOC]]), in_=gf_all[:])

    _split_multiwait(nc)
    return nc


# ----------------------------------------------------------------------------
# entry point
# ----------------------------------------------------------------------------
_CACHE = {}


def kernel(**inputs):
    shared = host_prep_shared(inputs)
    cores = host_prep_percore(inputs)

    if "nc" not in _CACHE:
        _CACHE["nc"] = build_nc(shared)
    nc = _CACHE["nc"]

    in_maps = []
    for ci in range(N_CORES):
        m = dict(cores[ci])
        for name, arr in shared.items():
            m[name] = arr
        in_maps.append(m)

    trace = bool(int(os.environ.get("AME2_TRACE", "0")))
    res = run_bass_kernel_spmd(nc, in_maps, core_ids=list(range(N_CORES)),
                               trace=trace)
    if trace and res.exec_time_ns is not None:
        _CACHE["exec_time_ns"] = res.exec_time_ns
    outs = [res.results[ci]["out"] for ci in range(N_CORES)]
    return np.concatenate(outs, axis=0).astype(np.float32)
